# revision 17
# baseline (speedup 1.0000x reference)
"""GNN message-passing (3x SAGEConv + global mean pool) on 8 TRN2 NeuronCores.

Sharding: nodes in 8 contiguous ranges of 6272 (50000 padded to 50176); each
core owns all edges whose dst lands in its range.  Aggregation per 128-node
dst tile via one-hot matmuls: aggT[f, n] = sum_chunks m_chunk.T @ S_chunk,
where m = gathered h[src] rows (dma_gather, bf16 tables) and S[e, n] =
(segid[e] == n) * inv_cnt[e] precomputed on the host (bf16, streamed from
HBM).  dma_gather indices are int16 (max 32768 rows per table), so node
features live in two region tables: region A = each core's first 4096 local
rows (tableA, 8*4096 = 32768 rows), region B = the remaining 2176 (tableB,
17408 rows).  Gather calls are capped at 1024 indices (SWDGE descriptor-ring
limit), round-robined over 4 SWDGE queues, and deep-buffered in call-sized
SBUF slots so Q7 descriptor generation runs ahead of the consuming matmuls.
Between layers each region is AllGathered separately: AG-A fires after the
first 32 dst tiles and overlaps the remaining tiles' compute; AG-B overlaps
the next layer's region-A gathers.
"""

import numpy as np
import ml_dtypes

# ---------------------------------------------------------------- constants
N_NODES = 50000
N_EDGES = 600000
D = 128
OUT = 40
NG = 128          # graphs
NCORES = 8
OWN = 6272        # nodes per core (padded)
NPAD = OWN * NCORES   # 50176
TILES = 49        # dst tiles per core (128 nodes each)
TILE_N = 128
RA = 4096         # region-A local rows per core (tiles 0..31)
RB = OWN - RA     # 2176 (tiles 32..48)
NROWS_A = RA * NCORES   # 32768
NROWS_B = RB * NCORES   # 17408
DOUTS = [128, 128, 40]

TABLE_NP = ml_dtypes.bfloat16   # gather-table / message dtype

import os
DBG_LAYERS = int(os.environ.get("DBG_LAYERS", "3"))
DBG_COLLECTIVE = int(os.environ.get("DBG_COLLECTIVE", "1"))
CALL_CH = 8        # 128-idx chunks per dma_gather call (ring limit 1024 idxs)
N_QUEUES = 4
S_BLOCK = 7        # tiles per S-stream block
LOOKAHEAD = 12     # tiles of gather-call lookahead


# ---------------------------------------------------------------- host side
def _row_of(src):
    """Map original node id -> (is_b, table_row)."""
    co = src // OWN
    li = src % OWN
    is_b = li >= RA
    row = np.where(is_b, co * RB + (li - RA), co * RA + li)
    return is_b, row


def _build_schedule(src, dst, inv_cnt):
    """Per-core edge schedule.  Returns (c_a, c_b, per-core dict arrays)."""
    order = np.argsort(dst, kind="stable")
    src_s = src[order]
    dst_s = dst[order]
    is_b_s, row_s = _row_of(src_s)
    bounds = np.searchsorted(dst_s, np.arange(0, NPAD + 1, TILE_N))

    tiles = []  # per global tile: (a_row, a_seg, a_scale, b_row, b_seg, b_scale)
    for g in range(NPAD // TILE_N):
        a, b = bounds[g], bounds[g + 1]
        r = row_s[a:b]
        seg = (dst_s[a:b] % TILE_N).astype(np.int32)
        scale = inv_cnt[dst_s[a:b]].astype(np.float32)
        m = is_b_s[a:b]
        tiles.append((r[~m], seg[~m], scale[~m], r[m], seg[m], scale[m]))

    c_a = max(1, max((len(t[0]) + 127) // 128 for t in tiles))
    c_b = max(1, max((len(t[3]) + 127) // 128 for t in tiles))
    cpt = c_a + c_b

    cores = []
    for c in range(NCORES):
        idx_a = np.zeros((TILES * c_a * 128,), np.int16)
        idx_b = np.zeros((TILES * c_b * 128,), np.int16)
        seg_arr = np.full((TILES * cpt, 128), 255, np.int32)
        scale_arr = np.zeros((TILES * cpt, 128), np.float32)
        for t in range(TILES):
            g = c * TILES + t
            a_r, a_g, a_v, b_r, b_g, b_v = tiles[g]
            for half, (cN, idx_arr, rows, segs, scales) in enumerate([
                (c_a, idx_a, a_r, a_g, a_v),
                (c_b, idx_b, b_r, b_g, b_v),
            ]):
                n = len(rows)
                idx_arr[t * cN * 128: t * cN * 128 + n] = rows.astype(np.int16)
                mbase = t * cpt + (c_a if half else 0)
                for j in range((n + 127) // 128):
                    lo_, hi_ = j * 128, min((j + 1) * 128, n)
                    seg_arr[mbase + j, 0:hi_ - lo_] = segs[lo_:hi_]
                    scale_arr[mbase + j, 0:hi_ - lo_] = scales[lo_:hi_]
        S = (seg_arr[:, :, None] == np.arange(128)[None, None, :])
        S = (S * scale_arr[:, :, None]).astype(TABLE_NP)        # [cols, 128, 128]
        S = np.ascontiguousarray(S.transpose(1, 0, 2))           # [128, cols, 128]
        wrap = lambda a: np.tile(np.ascontiguousarray(a.reshape(-1, 16).T), (8, 1))
        cores.append({
            "idx_a": wrap(idx_a),
            "idx_b": wrap(idx_b),
            "S": S,
        })
    return c_a, c_b, cores


def _build_nc(c_a, c_b):
    import concourse.bacc as bacc
    import concourse.bass as bass
    import concourse.mybir as mybir
    import concourse.tile as tile

    f32 = mybir.dt.float32
    TDT = mybir.dt.bfloat16
    CPT = c_a + c_b
    TCA = TILES * c_a
    TCB = TILES * c_b
    n_calls_a = (TCA + CALL_CH - 1) // CALL_CH
    n_calls_b = (TCB + CALL_CH - 1) // CALL_CH

    nc = bacc.Bacc("TRN2", target_bir_lowering=False, debug=False,
                   num_devices=NCORES, num_swdge_queues=N_QUEUES)

    # ---- I/O
    r0a = nc.dram_tensor("r0a", [NROWS_A, D], TDT, kind="ExternalInput")
    r0b = nc.dram_tensor("r0b", [NROWS_B, D], TDT, kind="ExternalInput")
    xT = nc.dram_tensor("xT", [D, OWN], TDT, kind="ExternalInput")
    idx_a_d = nc.dram_tensor("idx_a", [128, TCA * 8], mybir.dt.int16, kind="ExternalInput")
    idx_b_d = nc.dram_tensor("idx_b", [128, TCB * 8], mybir.dt.int16, kind="ExternalInput")
    S_d = nc.dram_tensor("S", [128, TILES * CPT, 128], TDT, kind="ExternalInput")
    G_d = nc.dram_tensor("G", [128, TILES, 128], f32, kind="ExternalInput")
    ident_d = nc.dram_tensor("ident", [128, 128], f32, kind="ExternalInput")
    identb_d = nc.dram_tensor("identb", [128, 128], TDT, kind="ExternalInput")
    w_d = {}
    for l in range(3):
        w_d[f"wl{l}"] = nc.dram_tensor(f"wl{l}", [D, DOUTS[l]], TDT, kind="ExternalInput")
        w_d[f"wr{l}"] = nc.dram_tensor(f"wr{l}", [D, DOUTS[l]], TDT, kind="ExternalInput")
        w_d[f"bl{l}"] = nc.dram_tensor(f"bl{l}", [DOUTS[l], 1], f32, kind="ExternalInput")
    out_h = nc.dram_tensor("out_h", [OWN, OUT], f32, kind="ExternalOutput")
    out_g = nc.dram_tensor("out_g", [OUT, NG], f32, kind="ExternalOutput")

    AluOp = mybir.AluOpType
    ActF = mybir.ActivationFunctionType

    with tile.TileContext(nc) as tc:
        with (
            tc.tile_pool(name="cst", bufs=1) as cst,
            tc.tile_pool(name="sb", bufs=1) as sb,
            tc.tile_pool(name="work", bufs=1) as work,
            tc.tile_pool(name="ps", bufs=1, space="PSUM") as ps,
            tc.tile_pool(name="dram", bufs=1, space="DRAM") as dram,
        ):
            # ---- constants to SBUF
            idxa_sb = cst.tile([128, TCA * 8], mybir.dt.int16)
            nc.sync.dma_start(idxa_sb[:], idx_a_d[:])
            idxb_sb = cst.tile([128, TCB * 8], mybir.dt.int16)
            nc.sync.dma_start(idxb_sb[:], idx_b_d[:])
            ident_sb = cst.tile([128, 128], f32)
            nc.sync.dma_start(ident_sb[:], ident_d[:])
            identb_sb = cst.tile([128, 128], TDT)
            nc.sync.dma_start(identb_sb[:], identb_d[:])
            w_sb = {}
            for l in range(3):
                for k in (f"wl{l}", f"wr{l}"):
                    w_sb[k] = cst.tile([D, DOUTS[l]], TDT, name=k)
                    nc.sync.dma_start(w_sb[k][:], w_d[k][:])
                w_sb[f"bl{l}"] = cst.tile([DOUTS[l], 1], f32, name=f"bl{l}")
                nc.sync.dma_start(w_sb[f"bl{l}"][:], w_d[f"bl{l}"][:])

            # ---- h buffers (feature-major)
            h0T = sb.tile([D, OWN], TDT)
            nc.sync.dma_start(h0T[:], xT[:])
            h1T = sb.tile([D, OWN], TDT)
            h2T = sb.tile([D, OWN], TDT)
            h3T = sb.tile([OUT, OWN], f32)
            hbufs = [h0T, h1T, h2T, h3T]

            # ---- DRAM bounce + region tables per layer boundary
            ag_in = [dram.tile([OWN, D], TDT, name=f"ag_in{i}") for i in range(2)]
            tabA = [dram.tile([NROWS_A, D], TDT, name=f"tabA{i}", addr_space="Shared")
                    for i in range(2)]
            tabB = [dram.tile([NROWS_B, D], TDT, name=f"tabB{i}", addr_space="Shared")
                    for i in range(2)]

            gT_ps = ps.tile([OUT, NG], f32, tag="gT", bufs=1)

            qrr = [0]
            for l in range(DBG_LAYERS):
                hT = hbufs[l]
                hnT = hbufs[l + 1]
                dout = DOUTS[l]
                tblA = r0a if l == 0 else tabA[l - 1]
                tblB = r0b if l == 0 else tabB[l - 1]

                slots_a = {}
                slots_b = {}
                next_a = [0]
                next_b = [0]
                s_blocks = {}

                def issue_calls(through_tile, l=l, tblA=tblA, tblB=tblB,
                                slots_a=slots_a, slots_b=slots_b,
                                next_a=next_a, next_b=next_b):
                    tgt_a = min((through_tile + 1) * c_a, TCA)
                    while next_a[0] * CALL_CH < tgt_a:
                        k = next_a[0]
                        a0 = k * CALL_CH
                        a1 = min(a0 + CALL_CH, TCA)
                        gt = work.tile([128, CALL_CH, D], TDT, tag="slotA",
                                       bufs=16, name=f"slotA_l{l}_{k}")
                        slots_a[k] = gt
                        n = (a1 - a0) * 128
                        nc.gpsimd.dma_gather(
                            gt[:, 0:a1 - a0, :], tblA[:, :],
                            idxa_sb[:, a0 * 8:a1 * 8], n, n, D, elem_step=D,
                            queue_num=qrr[0] % N_QUEUES)
                        qrr[0] += 1
                        next_a[0] += 1
                    tgt_b = min((through_tile + 1) * c_b, TCB)
                    while next_b[0] * CALL_CH < tgt_b:
                        k = next_b[0]
                        b0 = k * CALL_CH
                        b1 = min(b0 + CALL_CH, TCB)
                        gt = work.tile([128, CALL_CH, D], TDT, tag="slotB",
                                       bufs=10, name=f"slotB_l{l}_{k}")
                        slots_b[k] = gt
                        n = (b1 - b0) * 128
                        nc.gpsimd.dma_gather(
                            gt[:, 0:b1 - b0, :], tblB[:, :],
                            idxb_sb[:, b0 * 8:b1 * 8], n, n, D, elem_step=D,
                            queue_num=qrr[0] % N_QUEUES)
                        qrr[0] += 1
                        next_b[0] += 1

                for t in range(TILES):
                    if t % S_BLOCK == 0:
                        blk = t // S_BLOCK
                        nt = min(S_BLOCK, TILES - t)
                        S_sb = work.tile([128, S_BLOCK * CPT, 128], TDT,
                                         tag="S", bufs=2, name=f"S_l{l}_{blk}")
                        s_blocks[blk] = S_sb
                        nc.sync.dma_start(
                            S_sb[:, 0:nt * CPT, :],
                            S_d[:, t * CPT:(t + nt) * CPT, :])
                    issue_calls(min(t + LOOKAHEAD, TILES - 1))
                    S_sb = s_blocks[t // S_BLOCK]
                    tl = t % S_BLOCK

                    aggT = ps.tile([128, 128], f32, tag="aggT", bufs=2)
                    for j in range(CPT):
                        if j < c_a:
                            g = t * c_a + j
                            lhs = slots_a[g // CALL_CH][:, g % CALL_CH, :]
                        else:
                            g = t * c_b + (j - c_a)
                            lhs = slots_b[g // CALL_CH][:, g % CALL_CH, :]
                        nc.tensor.matmul(
                            out=aggT[:], lhsT=lhs,
                            rhs=S_sb[:, tl * CPT + j, :],
                            start=(j == 0), stop=(j == CPT - 1))
                    agg_sb = work.tile([128, 128], TDT, tag="agg_sb", bufs=3)
                    nc.scalar.copy(agg_sb[:], aggT[:])
                    lin = ps.tile([dout, 128], f32, tag="lin", bufs=2)
                    nc.tensor.matmul(out=lin[:], lhsT=w_sb[f"wl{l}"][:],
                                     rhs=agg_sb[:], start=True, stop=False)
                    nc.tensor.matmul(out=lin[:], lhsT=w_sb[f"wr{l}"][:],
                                     rhs=hT[:, t * 128:(t + 1) * 128],
                                     start=False, stop=True)
                    if l < 2:
                        nc.scalar.activation(
                            hnT[:, t * 128:(t + 1) * 128], lin[:],
                            ActF.Relu, bias=w_sb[f"bl{l}"][:])
                        # node-major transpose for the next layer's table
                        tp = ps.tile([128, 128], TDT, tag="tp", bufs=2)
                        nc.tensor.transpose(tp[:], hnT[:, t * 128:(t + 1) * 128],
                                            identb_sb[:])
                        tpsb = work.tile([128, 128], TDT, tag="tpsb", bufs=3)
                        nc.scalar.copy(tpsb[:], tp[:])
                        nc.sync.dma_start(ag_in[l][t * 128:(t + 1) * 128, :], tpsb[:])
                        if t == RA // 128 - 1:      # tiles 0..31 done -> region A
                            if DBG_COLLECTIVE:
                                nc.gpsimd.collective_compute(
                                    "AllGather", AluOp.bypass,
                                    replica_groups=[list(range(NCORES))],
                                    ins=[ag_in[l][0:RA, :].opt()],
                                    outs=[tabA[l][:].opt()])
                            else:
                                nc.sync.dma_start(tabA[l][0:RA, :], ag_in[l][0:RA, :])
                        if t == TILES - 1:          # region B
                            if DBG_COLLECTIVE:
                                nc.gpsimd.collective_compute(
                                    "AllGather", AluOp.bypass,
                                    replica_groups=[list(range(NCORES))],
                                    ins=[ag_in[l][RA:OWN, :].opt()],
                                    outs=[tabB[l][:].opt()])
                            else:
                                nc.sync.dma_start(tabB[l][0:RB, :], ag_in[l][RA:OWN, :])
                    else:
                        nc.vector.tensor_scalar_add(
                            hnT[:, t * 128:(t + 1) * 128], lin[:],
                            w_sb[f"bl{l}"][:])
                        tp = ps.tile([128, OUT], f32, tag="tp", bufs=2)
                        nc.tensor.transpose(tp[:], hnT[:, t * 128:(t + 1) * 128],
                                            ident_sb[0:OUT, 0:OUT])
                        tpsb = work.tile([128, OUT], f32, tag="tpsb2", bufs=3)
                        nc.scalar.copy(tpsb[:], tp[:])
                        nc.sync.dma_start(out_h[t * 128:(t + 1) * 128, :], tpsb[:])
                        G_sb = work.tile([128, 128], f32, tag="G", bufs=3)
                        nc.sync.dma_start(G_sb[:], G_d[:, t, :])
                        nc.tensor.matmul(out=gT_ps[:], lhsT=tpsb[:], rhs=G_sb[:],
                                         start=(t == 0), stop=(t == TILES - 1),
                                         skip_group_check=True)
                if l == 2:
                    g_sb = work.tile([OUT, NG], f32)
                    nc.scalar.copy(g_sb[:], gT_ps[:])
                    nc.sync.dma_start(out_g[:], g_sb[:])
    nc.compile()
    return nc


_CACHED = {}


def kernel(x, edge_index, batch, Wl0, bl0, Wr0, Wl1, bl1, Wr1, Wl2, bl2, Wr2):
    from concourse.bass_utils import run_bass_kernel_spmd

    x = np.asarray(x, np.float32)
    ei = np.asarray(edge_index, np.int64)
    batch_np = np.asarray(batch, np.int64)
    src, dst = ei[0], ei[1]

    cnt = np.bincount(dst, minlength=N_NODES).astype(np.float32)
    inv_cnt = (1.0 / np.maximum(cnt, 1.0)).astype(np.float32)
    inv_cnt_pad = np.zeros((NPAD,), np.float32)
    inv_cnt_pad[:N_NODES] = inv_cnt

    c_a, c_b, cores = _build_schedule(src, dst, inv_cnt_pad)

    # region tables for layer 0: relu(x) in table-row order
    rx = np.zeros((NPAD, D), np.float32)
    rx[:N_NODES] = np.maximum(x, 0.0)
    all_nodes = np.arange(NPAD)
    is_b, rows = _row_of(all_nodes)
    r0a = np.zeros((NROWS_A, D), TABLE_NP)
    r0b = np.zeros((NROWS_B, D), TABLE_NP)
    r0a[rows[~is_b]] = rx[all_nodes[~is_b]].astype(TABLE_NP)
    r0b[rows[is_b]] = rx[all_nodes[is_b]].astype(TABLE_NP)

    x_pad = np.zeros((NPAD, D), np.float32)
    x_pad[:N_NODES] = x

    gcnt = np.bincount(batch_np, minlength=NG).astype(np.float32)
    inv_g = (1.0 / np.maximum(gcnt, 1.0)).astype(np.float32)
    batch_pad = np.full((NPAD,), 255, np.int32)
    batch_pad[:N_NODES] = batch_np
    invg_pad = np.zeros((NPAD,), np.float32)
    invg_pad[:N_NODES] = inv_g[batch_np]

    ident = np.eye(128, dtype=np.float32)

    weights = {}
    for l, (Wl, bl, Wr) in enumerate([(Wl0, bl0, Wr0), (Wl1, bl1, Wr1), (Wl2, bl2, Wr2)]):
        weights[f"wl{l}"] = np.ascontiguousarray(np.asarray(Wl, np.float32).T).astype(TABLE_NP)
        weights[f"wr{l}"] = np.ascontiguousarray(np.asarray(Wr, np.float32).T).astype(TABLE_NP)
        weights[f"bl{l}"] = np.asarray(bl, np.float32).reshape(-1, 1)

    in_maps = []
    for c in range(NCORES):
        sl = slice(c * OWN, (c + 1) * OWN)
        bt = batch_pad[sl].reshape(TILES, 128)
        gv = invg_pad[sl].reshape(TILES, 128)
        G = (bt[:, :, None] == np.arange(128)[None, None, :])
        G = (G * gv[:, :, None]).astype(np.float32)
        G = np.ascontiguousarray(G.transpose(1, 0, 2))
        in_maps.append({
            "r0a": r0a,
            "r0b": r0b,
            "xT": np.ascontiguousarray(x_pad[sl].T).astype(TABLE_NP),
            "idx_a": cores[c]["idx_a"],
            "idx_b": cores[c]["idx_b"],
            "S": cores[c]["S"],
            "G": G,
            "ident": ident,
            "identb": ident.astype(TABLE_NP),
            **weights,
        })

    key = (c_a, c_b)
    if key not in _CACHED:
        _CACHED[key] = _build_nc(c_a, c_b)
    nc = _CACHED[key]

    res = run_bass_kernel_spmd(nc, in_maps, core_ids=list(range(NCORES)),
                               tmpdir=os.environ.get("KERNEL_PROFILE_DIR") or None)
    globals()["_LAST_RES"] = res

    h_full = np.zeros((N_NODES, OUT), np.float32)
    gT = np.zeros((OUT, NG), np.float32)
    for c in range(NCORES):
        a = c * OWN
        b = min((c + 1) * OWN, N_NODES)
        h_full[a:b] = res.results[c]["out_h"][:b - a]
        gT += res.results[c]["out_g"]
    return h_full, np.ascontiguousarray(gT.T)


if __name__ == "__main__":
    import jax
    import reference
    cpu = jax.devices("cpu")[0]
    with jax.default_device(cpu):
        inputs = {k: np.asarray(v) for k, v in reference.setup_inputs().items()}
        eh, eg = reference.reference(**inputs)
        eh, eg = np.asarray(eh), np.asarray(eg)
    import time
    t0 = time.time()
    h, g = kernel(**inputs)
    print(f"kernel wall time: {time.time() - t0:.1f}s")
    def relerr(a, b):
        return np.abs(a - b).max() / (np.abs(b).max() + 1e-12)
    print("h rel err:", relerr(h, eh))
    print("g rel err:", relerr(g, eg))


# revision 18
# speedup vs baseline: 1.0103x; 1.0103x over previous
"""GNN message-passing (3x SAGEConv + global mean pool) on 8 TRN2 NeuronCores.

Sharding: nodes in 8 contiguous ranges of 6272 (50000 padded to 50176); each
core owns all edges whose dst lands in its range.  Aggregation per 128-node
dst tile via one-hot matmuls: aggT[f, n] = sum_chunks m_chunk.T @ S_chunk,
where m = gathered h[src] rows (dma_gather, bf16 tables) and S[e, n] =
(segid[e] == n) * inv_cnt[e] precomputed on the host (bf16, streamed from
HBM).  dma_gather indices are int16 (max 32768 rows per table), so node
features live in two region tables: region A = each core's first 4096 local
rows (tableA, 8*4096 = 32768 rows), region B = the remaining 2176 (tableB,
17408 rows).  Gather calls are capped at 1024 indices (SWDGE descriptor-ring
limit), round-robined over 4 SWDGE queues, and deep-buffered in call-sized
SBUF slots so Q7 descriptor generation runs ahead of the consuming matmuls.
Between layers each region is AllGathered separately: AG-A fires after the
first 32 dst tiles and overlaps the remaining tiles' compute; AG-B overlaps
the next layer's region-A gathers.
"""

import numpy as np
import ml_dtypes

# ---------------------------------------------------------------- constants
N_NODES = 50000
N_EDGES = 600000
D = 128
OUT = 40
NG = 128          # graphs
NCORES = 8
OWN = 6272        # nodes per core (padded)
NPAD = OWN * NCORES   # 50176
TILES = 49        # dst tiles per core (128 nodes each)
TILE_N = 128
RA = 4096         # region-A local rows per core (tiles 0..31)
RB = OWN - RA     # 2176 (tiles 32..48)
NROWS_A = RA * NCORES   # 32768
NROWS_B = RB * NCORES   # 17408
DOUTS = [128, 128, 40]

TABLE_NP = ml_dtypes.bfloat16   # gather-table / message dtype

import os
DBG_LAYERS = int(os.environ.get("DBG_LAYERS", "3"))
DBG_COLLECTIVE = int(os.environ.get("DBG_COLLECTIVE", "1"))
CALL_CH = 8        # 128-idx chunks per dma_gather call (ring limit 1024 idxs)
N_QUEUES = 4
S_BLOCK = 4        # tiles per S-stream block
LOOKAHEAD = 12     # tiles of gather-call lookahead


# ---------------------------------------------------------------- host side
def _row_of(src):
    """Map original node id -> (is_b, table_row)."""
    co = src // OWN
    li = src % OWN
    is_b = li >= RA
    row = np.where(is_b, co * RB + (li - RA), co * RA + li)
    return is_b, row


def _build_schedule(src, dst, inv_cnt):
    """Per-core edge schedule.  Returns (c_a, c_b, per-core dict arrays)."""
    order = np.argsort(dst, kind="stable")
    src_s = src[order]
    dst_s = dst[order]
    is_b_s, row_s = _row_of(src_s)
    bounds = np.searchsorted(dst_s, np.arange(0, NPAD + 1, TILE_N))

    tiles = []  # per global tile: (a_row, a_seg, a_scale, b_row, b_seg, b_scale)
    for g in range(NPAD // TILE_N):
        a, b = bounds[g], bounds[g + 1]
        r = row_s[a:b]
        seg = (dst_s[a:b] % TILE_N).astype(np.int32)
        scale = inv_cnt[dst_s[a:b]].astype(np.float32)
        m = is_b_s[a:b]
        tiles.append((r[~m], seg[~m], scale[~m], r[m], seg[m], scale[m]))

    c_a = max(1, max((len(t[0]) + 127) // 128 for t in tiles))
    c_b = max(1, max((len(t[3]) + 127) // 128 for t in tiles))
    cpt = c_a + c_b

    cores = []
    for c in range(NCORES):
        idx_a = np.zeros((TILES * c_a * 128,), np.int16)
        idx_b = np.zeros((TILES * c_b * 128,), np.int16)
        seg_arr = np.full((TILES * cpt, 128), 255, np.int32)
        scale_arr = np.zeros((TILES * cpt, 128), np.float32)
        for t in range(TILES):
            g = c * TILES + t
            a_r, a_g, a_v, b_r, b_g, b_v = tiles[g]
            for half, (cN, idx_arr, rows, segs, scales) in enumerate([
                (c_a, idx_a, a_r, a_g, a_v),
                (c_b, idx_b, b_r, b_g, b_v),
            ]):
                n = len(rows)
                idx_arr[t * cN * 128: t * cN * 128 + n] = rows.astype(np.int16)
                mbase = t * cpt + (c_a if half else 0)
                for j in range((n + 127) // 128):
                    lo_, hi_ = j * 128, min((j + 1) * 128, n)
                    seg_arr[mbase + j, 0:hi_ - lo_] = segs[lo_:hi_]
                    scale_arr[mbase + j, 0:hi_ - lo_] = scales[lo_:hi_]
        S = (seg_arr[:, :, None] == np.arange(128)[None, None, :])
        S = (S * scale_arr[:, :, None]).astype(TABLE_NP)        # [cols, 128, 128]
        S = np.ascontiguousarray(S.transpose(1, 0, 2))           # [128, cols, 128]
        wrap = lambda a: np.tile(np.ascontiguousarray(a.reshape(-1, 16).T), (8, 1))
        cores.append({
            "idx_a": wrap(idx_a),
            "idx_b": wrap(idx_b),
            "S": S,
        })
    return c_a, c_b, cores


def _build_nc(c_a, c_b):
    import concourse.bacc as bacc
    import concourse.bass as bass
    import concourse.mybir as mybir
    import concourse.tile as tile

    f32 = mybir.dt.float32
    TDT = mybir.dt.bfloat16
    CPT = c_a + c_b
    TCA = TILES * c_a
    TCB = TILES * c_b
    n_calls_a = (TCA + CALL_CH - 1) // CALL_CH
    n_calls_b = (TCB + CALL_CH - 1) // CALL_CH

    nc = bacc.Bacc("TRN2", target_bir_lowering=False, debug=False,
                   num_devices=NCORES, num_swdge_queues=N_QUEUES)

    # ---- I/O
    r0a = nc.dram_tensor("r0a", [NROWS_A, D], TDT, kind="ExternalInput")
    r0b = nc.dram_tensor("r0b", [NROWS_B, D], TDT, kind="ExternalInput")
    xT = nc.dram_tensor("xT", [D, OWN], TDT, kind="ExternalInput")
    idx_a_d = nc.dram_tensor("idx_a", [128, TCA * 8], mybir.dt.int16, kind="ExternalInput")
    idx_b_d = nc.dram_tensor("idx_b", [128, TCB * 8], mybir.dt.int16, kind="ExternalInput")
    S_d = nc.dram_tensor("S", [128, TILES * CPT, 128], TDT, kind="ExternalInput")
    G_d = nc.dram_tensor("G", [128, TILES, 128], f32, kind="ExternalInput")
    ident_d = nc.dram_tensor("ident", [128, 128], f32, kind="ExternalInput")
    identb_d = nc.dram_tensor("identb", [128, 128], TDT, kind="ExternalInput")
    w_d = {}
    for l in range(3):
        w_d[f"wl{l}"] = nc.dram_tensor(f"wl{l}", [D, DOUTS[l]], TDT, kind="ExternalInput")
        w_d[f"wr{l}"] = nc.dram_tensor(f"wr{l}", [D, DOUTS[l]], TDT, kind="ExternalInput")
        w_d[f"bl{l}"] = nc.dram_tensor(f"bl{l}", [DOUTS[l], 1], f32, kind="ExternalInput")
    out_h = nc.dram_tensor("out_h", [OWN, OUT], f32, kind="ExternalOutput")
    out_g = nc.dram_tensor("out_g", [OUT, NG], f32, kind="ExternalOutput")

    AluOp = mybir.AluOpType
    ActF = mybir.ActivationFunctionType

    with tile.TileContext(nc) as tc:
        with (
            tc.tile_pool(name="cst", bufs=1) as cst,
            tc.tile_pool(name="sb", bufs=1) as sb,
            tc.tile_pool(name="work", bufs=1) as work,
            tc.tile_pool(name="ps", bufs=1, space="PSUM") as ps,
            tc.tile_pool(name="dram", bufs=1, space="DRAM") as dram,
        ):
            # ---- constants to SBUF
            idxa_sb = cst.tile([128, TCA * 8], mybir.dt.int16)
            nc.sync.dma_start(idxa_sb[:], idx_a_d[:])
            idxb_sb = cst.tile([128, TCB * 8], mybir.dt.int16)
            nc.sync.dma_start(idxb_sb[:], idx_b_d[:])
            ident_sb = cst.tile([128, 128], f32)
            nc.sync.dma_start(ident_sb[:], ident_d[:])
            identb_sb = cst.tile([128, 128], TDT)
            nc.sync.dma_start(identb_sb[:], identb_d[:])
            w_sb = {}
            for l in range(3):
                for k in (f"wl{l}", f"wr{l}"):
                    w_sb[k] = cst.tile([D, DOUTS[l]], TDT, name=k)
                    nc.sync.dma_start(w_sb[k][:], w_d[k][:])
                w_sb[f"bl{l}"] = cst.tile([DOUTS[l], 1], f32, name=f"bl{l}")
                nc.sync.dma_start(w_sb[f"bl{l}"][:], w_d[f"bl{l}"][:])

            # ---- h buffers (feature-major)
            h0T = sb.tile([D, OWN], TDT)
            nc.sync.dma_start(h0T[:], xT[:])
            h1T = sb.tile([D, OWN], TDT)
            h2T = sb.tile([D, OWN], TDT)
            h3T = sb.tile([OUT, OWN], f32)
            hbufs = [h0T, h1T, h2T, h3T]
            aggbuf = sb.tile([D, OWN], TDT)

            # ---- DRAM bounce + region tables per layer boundary
            ag_in = [dram.tile([OWN, D], TDT, name=f"ag_in{i}") for i in range(2)]
            tabA = [dram.tile([NROWS_A, D], TDT, name=f"tabA{i}", addr_space="Shared")
                    for i in range(2)]
            tabB = [dram.tile([NROWS_B, D], TDT, name=f"tabB{i}", addr_space="Shared")
                    for i in range(2)]

            gT_ps = ps.tile([OUT, NG], f32, tag="gT", bufs=1)

            qrr = [0]
            for l in range(DBG_LAYERS):
                hT = hbufs[l]
                hnT = hbufs[l + 1]
                dout = DOUTS[l]
                tblA = r0a if l == 0 else tabA[l - 1]
                tblB = r0b if l == 0 else tabB[l - 1]

                slots_a = {}
                slots_b = {}
                next_a = [0]
                next_b = [0]
                s_blocks = {}

                def issue_calls(through_tile, l=l, tblA=tblA, tblB=tblB,
                                slots_a=slots_a, slots_b=slots_b,
                                next_a=next_a, next_b=next_b):
                    tgt_a = min((through_tile + 1) * c_a, TCA)
                    while next_a[0] * CALL_CH < tgt_a:
                        k = next_a[0]
                        a0 = k * CALL_CH
                        a1 = min(a0 + CALL_CH, TCA)
                        gt = work.tile([128, CALL_CH, D], TDT, tag="slotA",
                                       bufs=12, name=f"slotA_l{l}_{k}")
                        slots_a[k] = gt
                        n = (a1 - a0) * 128
                        nc.gpsimd.dma_gather(
                            gt[:, 0:a1 - a0, :], tblA[:, :],
                            idxa_sb[:, a0 * 8:a1 * 8], n, n, D, elem_step=D,
                            queue_num=qrr[0] % N_QUEUES)
                        qrr[0] += 1
                        next_a[0] += 1
                    tgt_b = min((through_tile + 1) * c_b, TCB)
                    while next_b[0] * CALL_CH < tgt_b:
                        k = next_b[0]
                        b0 = k * CALL_CH
                        b1 = min(b0 + CALL_CH, TCB)
                        gt = work.tile([128, CALL_CH, D], TDT, tag="slotB",
                                       bufs=8, name=f"slotB_l{l}_{k}")
                        slots_b[k] = gt
                        n = (b1 - b0) * 128
                        nc.gpsimd.dma_gather(
                            gt[:, 0:b1 - b0, :], tblB[:, :],
                            idxb_sb[:, b0 * 8:b1 * 8], n, n, D, elem_step=D,
                            queue_num=qrr[0] % N_QUEUES)
                        qrr[0] += 1
                        next_b[0] += 1

                for t in range(TILES):
                    if t % S_BLOCK == 0:
                        blk = t // S_BLOCK
                        nt = min(S_BLOCK, TILES - t)
                        S_sb = work.tile([128, S_BLOCK * CPT, 128], TDT,
                                         tag="S", bufs=3, name=f"S_l{l}_{blk}")
                        s_blocks[blk] = S_sb
                        nc.sync.dma_start(
                            S_sb[:, 0:nt * CPT, :],
                            S_d[:, t * CPT:(t + nt) * CPT, :])
                    issue_calls(min(t + LOOKAHEAD, TILES - 1))
                    S_sb = s_blocks[t // S_BLOCK]
                    tl = t % S_BLOCK

                    aggT = ps.tile([128, 128], f32, tag="aggT", bufs=2)
                    for j in range(CPT):
                        if j < c_a:
                            g = t * c_a + j
                            lhs = slots_a[g // CALL_CH][:, g % CALL_CH, :]
                        else:
                            g = t * c_b + (j - c_a)
                            lhs = slots_b[g // CALL_CH][:, g % CALL_CH, :]
                        nc.tensor.matmul(
                            out=aggT[:], lhsT=lhs,
                            rhs=S_sb[:, tl * CPT + j, :],
                            start=(j == 0), stop=(j == CPT - 1))
                    nc.scalar.copy(aggbuf[:, t * 128:(t + 1) * 128], aggT[:])

                    # grouped linear: every 4 tiles (512 cols), stationary reused
                    if t % 4 == 3 or t == TILES - 1:
                        g0 = (t // 4) * 4
                        nct = (t - g0 + 1) * 128
                        cs = slice(g0 * 128, g0 * 128 + nct)
                        lin = ps.tile([dout, 512], f32, tag="lin", bufs=2)
                        nc.tensor.matmul(out=lin[:, 0:nct], lhsT=w_sb[f"wl{l}"][:],
                                         rhs=aggbuf[:, cs], start=True, stop=False)
                        nc.tensor.matmul(out=lin[:, 0:nct], lhsT=w_sb[f"wr{l}"][:],
                                         rhs=hT[:, cs], start=False, stop=True)
                        if l < 2:
                            nc.scalar.activation(
                                hnT[:, cs], lin[:, 0:nct],
                                ActF.Relu, bias=w_sb[f"bl{l}"][:])
                        else:
                            nc.vector.tensor_scalar_add(
                                hnT[:, cs], lin[:, 0:nct], w_sb[f"bl{l}"][:])
                        for tt in range(g0, t + 1):
                            if l < 2:
                                tp = ps.tile([128, 128], TDT, tag="tp", bufs=2)
                                nc.tensor.transpose(
                                    tp[:], hnT[:, tt * 128:(tt + 1) * 128],
                                    identb_sb[:])
                                tpsb = work.tile([128, 128], TDT, tag="tpsb", bufs=3)
                                nc.scalar.copy(tpsb[:], tp[:])
                                nc.sync.dma_start(
                                    ag_in[l][tt * 128:(tt + 1) * 128, :], tpsb[:])
                            else:
                                tp = ps.tile([128, OUT], f32, tag="tp", bufs=2)
                                nc.tensor.transpose(
                                    tp[:], hnT[:, tt * 128:(tt + 1) * 128],
                                    ident_sb[0:OUT, 0:OUT])
                                tpsb = work.tile([128, OUT], f32, tag="tpsb2", bufs=3)
                                nc.scalar.copy(tpsb[:], tp[:])
                                nc.sync.dma_start(
                                    out_h[tt * 128:(tt + 1) * 128, :], tpsb[:])
                                G_sb = work.tile([128, 128], f32, tag="G", bufs=3)
                                nc.sync.dma_start(G_sb[:], G_d[:, tt, :])
                                nc.tensor.matmul(
                                    out=gT_ps[:], lhsT=tpsb[:], rhs=G_sb[:],
                                    start=(tt == 0), stop=(tt == TILES - 1),
                                    skip_group_check=True)
                    if l < 2:
                        if t == RA // 128 - 1:      # tiles 0..31 done -> region A
                            if DBG_COLLECTIVE:
                                nc.gpsimd.collective_compute(
                                    "AllGather", AluOp.bypass,
                                    replica_groups=[list(range(NCORES))],
                                    ins=[ag_in[l][0:RA, :].opt()],
                                    outs=[tabA[l][:].opt()])
                            else:
                                nc.sync.dma_start(tabA[l][0:RA, :], ag_in[l][0:RA, :])
                        if t == TILES - 1:          # region B
                            if DBG_COLLECTIVE:
                                nc.gpsimd.collective_compute(
                                    "AllGather", AluOp.bypass,
                                    replica_groups=[list(range(NCORES))],
                                    ins=[ag_in[l][RA:OWN, :].opt()],
                                    outs=[tabB[l][:].opt()])
                            else:
                                nc.sync.dma_start(tabB[l][0:RB, :], ag_in[l][RA:OWN, :])
                if l == 2:
                    g_sb = work.tile([OUT, NG], f32)
                    nc.scalar.copy(g_sb[:], gT_ps[:])
                    nc.sync.dma_start(out_g[:], g_sb[:])
    nc.compile()
    return nc


_CACHED = {}


def kernel(x, edge_index, batch, Wl0, bl0, Wr0, Wl1, bl1, Wr1, Wl2, bl2, Wr2):
    from concourse.bass_utils import run_bass_kernel_spmd

    x = np.asarray(x, np.float32)
    ei = np.asarray(edge_index, np.int64)
    batch_np = np.asarray(batch, np.int64)
    src, dst = ei[0], ei[1]

    cnt = np.bincount(dst, minlength=N_NODES).astype(np.float32)
    inv_cnt = (1.0 / np.maximum(cnt, 1.0)).astype(np.float32)
    inv_cnt_pad = np.zeros((NPAD,), np.float32)
    inv_cnt_pad[:N_NODES] = inv_cnt

    c_a, c_b, cores = _build_schedule(src, dst, inv_cnt_pad)

    # region tables for layer 0: relu(x) in table-row order
    rx = np.zeros((NPAD, D), np.float32)
    rx[:N_NODES] = np.maximum(x, 0.0)
    all_nodes = np.arange(NPAD)
    is_b, rows = _row_of(all_nodes)
    r0a = np.zeros((NROWS_A, D), TABLE_NP)
    r0b = np.zeros((NROWS_B, D), TABLE_NP)
    r0a[rows[~is_b]] = rx[all_nodes[~is_b]].astype(TABLE_NP)
    r0b[rows[is_b]] = rx[all_nodes[is_b]].astype(TABLE_NP)

    x_pad = np.zeros((NPAD, D), np.float32)
    x_pad[:N_NODES] = x

    gcnt = np.bincount(batch_np, minlength=NG).astype(np.float32)
    inv_g = (1.0 / np.maximum(gcnt, 1.0)).astype(np.float32)
    batch_pad = np.full((NPAD,), 255, np.int32)
    batch_pad[:N_NODES] = batch_np
    invg_pad = np.zeros((NPAD,), np.float32)
    invg_pad[:N_NODES] = inv_g[batch_np]

    ident = np.eye(128, dtype=np.float32)

    weights = {}
    for l, (Wl, bl, Wr) in enumerate([(Wl0, bl0, Wr0), (Wl1, bl1, Wr1), (Wl2, bl2, Wr2)]):
        weights[f"wl{l}"] = np.ascontiguousarray(np.asarray(Wl, np.float32).T).astype(TABLE_NP)
        weights[f"wr{l}"] = np.ascontiguousarray(np.asarray(Wr, np.float32).T).astype(TABLE_NP)
        weights[f"bl{l}"] = np.asarray(bl, np.float32).reshape(-1, 1)

    in_maps = []
    for c in range(NCORES):
        sl = slice(c * OWN, (c + 1) * OWN)
        bt = batch_pad[sl].reshape(TILES, 128)
        gv = invg_pad[sl].reshape(TILES, 128)
        G = (bt[:, :, None] == np.arange(128)[None, None, :])
        G = (G * gv[:, :, None]).astype(np.float32)
        G = np.ascontiguousarray(G.transpose(1, 0, 2))
        in_maps.append({
            "r0a": r0a,
            "r0b": r0b,
            "xT": np.ascontiguousarray(x_pad[sl].T).astype(TABLE_NP),
            "idx_a": cores[c]["idx_a"],
            "idx_b": cores[c]["idx_b"],
            "S": cores[c]["S"],
            "G": G,
            "ident": ident,
            "identb": ident.astype(TABLE_NP),
            **weights,
        })

    key = (c_a, c_b)
    if key not in _CACHED:
        _CACHED[key] = _build_nc(c_a, c_b)
    nc = _CACHED[key]

    res = run_bass_kernel_spmd(nc, in_maps, core_ids=list(range(NCORES)),
                               tmpdir=os.environ.get("KERNEL_PROFILE_DIR") or None)
    globals()["_LAST_RES"] = res

    h_full = np.zeros((N_NODES, OUT), np.float32)
    gT = np.zeros((OUT, NG), np.float32)
    for c in range(NCORES):
        a = c * OWN
        b = min((c + 1) * OWN, N_NODES)
        h_full[a:b] = res.results[c]["out_h"][:b - a]
        gT += res.results[c]["out_g"]
    return h_full, np.ascontiguousarray(gT.T)


if __name__ == "__main__":
    import jax
    import reference
    cpu = jax.devices("cpu")[0]
    with jax.default_device(cpu):
        inputs = {k: np.asarray(v) for k, v in reference.setup_inputs().items()}
        eh, eg = reference.reference(**inputs)
        eh, eg = np.asarray(eh), np.asarray(eg)
    import time
    t0 = time.time()
    h, g = kernel(**inputs)
    print(f"kernel wall time: {time.time() - t0:.1f}s")
    def relerr(a, b):
        return np.abs(a - b).max() / (np.abs(b).max() + 1e-12)
    print("h rel err:", relerr(h, eh))
    print("g rel err:", relerr(g, eg))
